# revision 6
# baseline (speedup 1.0000x reference)
"""Additive-attention pooling kernel for 8 TRN2 NeuronCores.

reference:
    h     = tanh(x @ (W1+W2) + (b1+b2))      x: [B, T, D]
    score = h @ V + V_b                      [B, T, 1]
    attn  = softmax(score, axis=T)
    out   = sum_t attn * x                   [B, D]

Sharding: data-parallel over batch; each of the 8 cores gets B/8 = 8
examples (8 MB of fp32), weights replicated. No collectives.

Layout: token t = c*2048 + p*16 + i lands on partition p, tile i of
example c (token order within an example is permuted vs the reference;
softmax pooling is permutation-invariant).  Each partition reads
contiguous 2-8 KB runs from HBM.

v2 data path (vs the v1 SWDGE-cast design): x streams as plain f32
over the HARDWARE DGE (SP + ACT queues), which starts at ~4 us and
runs at HBM line rate, instead of the software-DGE cast path that was
desc-gen paced (~288 GB/s) and started at ~9 us behind the Q7
preamble.  f32 -> bf16 casts are split across DVE (c0, c1, part of
c7) and the otherwise-idle GPSIMD/Pool engine (c2..c6, rest of c7).
Identities are built on DVE so Q7 never blocks anything.

Per-core compute per 4-tile cluster: PE transposes x (xT) via the
identity; PE h-matmul with stationary Wc -> psum; ACT tanh(+bias) ->
hT sbuf; PE score matmul (stationary hT tile, 1-col rhs v) -> psum
column; ACT exp per example -> masked e-block column with accum_out
denominator; PE context matmul (stationary x tile, 8-col e-block rhs)
accumulating ctx^T [d, 8] across all 128 tiles in one psum bank.
PE emission is software-pipelined: transposes+h-matmuls of example c
run before scores of c (hiding the tanh latency), and ctx of c-1 is
interleaved into c's slot.  First and last examples run at 4-tile
granularity for ramp/tail latency; the middle at 8-tile.

V_b is omitted: softmax(score + c) == softmax(score) exactly.
Softmax runs without max-subtraction: |score| <= sum|V_u| ~ 9.
"""

import os

import numpy as np

B, T, D, U = 64, 2048, 128, 128
N_CORES = 8
B_LOC = B // N_CORES          # 8 examples per core
N_TILE_EX = T // 128          # 16 token-tiles per example
N_TILES = B_LOC * N_TILE_EX   # 128 tiles per core

_nc = None
LAST_RESULT = None


def _build_nc():
    import concourse.bass as bass  # noqa: F401
    import concourse.mybir as mybir
    import concourse.tile as tile
    from concourse import bacc

    f32 = mybir.dt.float32
    bf16 = mybir.dt.bfloat16
    ACT = mybir.ActivationFunctionType
    ALU = mybir.AluOpType

    nc = bacc.Bacc("TRN2", target_bir_lowering=False, debug=False,
                   num_devices=N_CORES)

    x_d = nc.declare_dram_parameter("encoder_outputs", [B_LOC * T, D], f32,
                                    isOutput=False)
    w1_d = nc.declare_dram_parameter("W1_w", [D, U], f32, isOutput=False)
    b1_d = nc.declare_dram_parameter("W1_b", [U, 1], f32, isOutput=False)
    w2_d = nc.declare_dram_parameter("W2_w", [D, U], f32, isOutput=False)
    b2_d = nc.declare_dram_parameter("W2_b", [U, 1], f32, isOutput=False)
    v_d = nc.declare_dram_parameter("V_w", [U, 1], f32, isOutput=False)
    out_d = nc.declare_dram_parameter("out", [B_LOC, D], f32, isOutput=True)

    # token = c*T + p*16 + i  ->  [c][p][i][d]; per-(c,p) HBM runs are 8 KB
    x_r = x_d.ap().rearrange("(c p i) d -> c p i d", c=B_LOC, p=128,
                             i=N_TILE_EX)

    def make_ident(eng, dst):
        eng.memset(dst, 0.0)
        eng.affine_select(out=dst, in_=dst, compare_op=ALU.not_equal,
                          fill=1.0, base=0, pattern=[[-1, dst.shape[1]]],
                          channel_multiplier=1)

    with tile.TileContext(nc) as tc:
        with (
            tc.tile_pool(name="consts", bufs=1) as consts,
            tc.tile_pool(name="big", bufs=1) as big,
            tc.tile_pool(name="ps_xt", bufs=2, space="PSUM") as ps_xt_pool,
            tc.tile_pool(name="ps_h", bufs=2, space="PSUM") as ps_h_pool,
            tc.tile_pool(name="ps_sc", bufs=1, space="PSUM") as ps_sc_pool,
            tc.tile_pool(name="ps_cx", bufs=1, space="PSUM") as ps_cx_pool,
        ):
            # ---- persistent buffers ----
            xf = big.tile([128, N_TILES, 128], f32)         # 8 MB staging
            x_bf = big.tile([128, N_TILES * 128], bf16)     # 4 MB  [t, d]
            xT = big.tile([128, N_TILES * 128], bf16)       # 4 MB  [d, t]
            ht = big.tile([128, N_TILES * 128], bf16)       # 4 MB tanh(h)^T
            eb = big.tile([128, N_TILES * 8], bf16)         # masked e-blocks
            e_all = big.tile([128, 8], f32)
            e_7a = big.tile([128, 1], f32)
            e_7b = big.tile([128, 1], f32)
            cxT_sb = big.tile([128, 8], f32)
            out_sb = big.tile([B_LOC, 128], f32)
            den_r = big.tile([B_LOC, 1], f32)

            x_bf_r = x_bf.rearrange("p (j d) -> p j d", d=128)
            eb_r = eb.rearrange("p (j e) -> p j e", e=8)

            ident = consts.tile([128, 128], bf16)
            ident_f = consts.tile([128, 128], f32)
            w1_sb = consts.tile([128, 128], f32)
            w2_sb = consts.tile([128, 128], f32)
            wc_bf = consts.tile([128, 128], bf16)   # [d, u] stationary
            b1_sb = consts.tile([128, 1], f32)
            b2_sb = consts.tile([128, 1], f32)
            b_sum = consts.tile([128, 1], f32)      # per-partition (=u) bias
            vf_sb = consts.tile([128, 1], f32)
            v_bf = consts.tile([128, 1], bf16)
            ones_col = consts.tile([128, 1], f32)
            act_warm = consts.tile([128, 1], f32)

            ps_sc = ps_sc_pool.tile([128, 128], f32)   # score col per tile
            ps_cx = ps_cx_pool.tile([128, 512], f32)

            # ================= DMA issue (all HWDGE, f32) =================
            # ACT-queue DGE: example 0 in quarters (fast compute ramp), then
            # the activation-table warm (exp/tanh share one table set).
            for q in range(4):
                nc.scalar.dma_start(out=xf[:, 4 * q:4 * (q + 1)],
                                    in_=x_r[0][:, 4 * q:4 * (q + 1)])
            # SP-queue DGE: weights early (wc needed ~first h-matmul), then
            # the bulk examples, last example in quarters (short tail).
            nc.sync.dma_start(out=w1_sb, in_=w1_d.ap())
            nc.sync.dma_start(out=w2_sb, in_=w2_d.ap())
            for c in range(1, 7):
                nc.sync.dma_start(out=xf[:, 16 * c:16 * (c + 1)],
                                  in_=x_r[c])
                if c == 1:
                    nc.sync.dma_start(out=b1_sb, in_=b1_d.ap())
                    nc.sync.dma_start(out=b2_sb, in_=b2_d.ap())
                elif c == 2:
                    nc.sync.dma_start(out=vf_sb, in_=v_d.ap())
            for q in range(4):
                nc.sync.dma_start(out=xf[:, 112 + 4 * q:112 + 4 * (q + 1)],
                                  in_=x_r[7][:, 4 * q:4 * (q + 1)])

            # ---- preamble: identities on the otherwise-idle Q7, weights
            # prep + c0/c1 casts on DVE ----
            make_ident(nc.gpsimd, ident)
            make_ident(nc.gpsimd, ident_f)   # needed only at the tail
            nc.vector.memset(ones_col, 1.0)
            # ACT: warm the exp/tanh table during the DMA ramp
            nc.scalar.activation(act_warm, ones_col, ACT.Exp)

            def cast(eng, lo, hi):     # xf -> x_bf, tile range [lo, hi)
                eng.tensor_copy(x_bf_r[:, lo:hi], xf[:, lo:hi])

            cast(nc.vector, 0, 4)
            nc.vector.tensor_add(wc_bf, w1_sb, w2_sb)
            cast(nc.vector, 4, 8)
            cast(nc.vector, 8, 12)
            nc.vector.tensor_add(b_sum, b1_sb, b2_sb)
            cast(nc.vector, 12, 16)
            nc.vector.tensor_copy(v_bf, vf_sb)
            nc.vector.memset(eb, 0.0)
            cast(nc.vector, 16, 24)          # c1 first half
            cast(nc.vector, 24, 32)          # c1 second half

            # GPSIMD (Pool) casts for c2..c6 in halves
            for c in range(2, 7):
                cast(nc.gpsimd, 16 * c, 16 * c + 8)
                cast(nc.gpsimd, 16 * c + 8, 16 * (c + 1))
            # c7 quarters alternate DVE (emitted later, in the main loop
            # position) / GPSIMD
            cast(nc.gpsimd, 116, 120)
            cast(nc.gpsimd, 124, 128)

            # ================= main pipeline =================
            # Per example c: PE does transposes + h-matmuls for both halves,
            # then ctx(c-1), then scores(c).  ACT does exp(c-1) then tanh(c).
            def transposes(j0, n):
                """n 4-tile clusters starting at tile j0 -> xT via psum."""
                for q in range(n):
                    pxt = ps_xt_pool.tile([128, 512], bf16, tag="pxt")
                    for r in range(4):
                        j = j0 + 4 * q + r
                        nc.tensor.transpose(
                            pxt[:, 128 * r:128 * (r + 1)],
                            x_bf_r[:, j], ident)
                    s = 128 * (j0 + 4 * q)
                    nc.vector.tensor_copy(xT[:, s:s + 512], pxt)

            def ctx_mms(c):
                for i in range(N_TILE_EX):
                    j = 16 * c + i
                    nc.tensor.matmul(ps_cx[:, 0:8], lhsT=x_bf_r[:, j],
                                     rhs=eb_r[:, j],
                                     start=(j == 0), stop=(j == N_TILES - 1))

            def scores(j0, n):
                for i in range(n):
                    j = j0 + i
                    nc.tensor.matmul(ps_sc[:, j:j + 1],
                                     lhsT=ht[:, 128 * j:128 * (j + 1)],
                                     rhs=v_bf)

            def exp_c(c):
                if c < B_LOC - 1:
                    nc.scalar.activation(eb_r[:, 16 * c:16 * c + 16, c],
                                         ps_sc[:, 16 * c:16 * c + 16],
                                         ACT.Exp,
                                         accum_out=e_all[:, c:c + 1])
                else:
                    nc.scalar.activation(eb_r[:, 16 * c:16 * c + 8, c],
                                         ps_sc[:, 16 * c:16 * c + 8],
                                         ACT.Exp, accum_out=e_7a)
                    nc.scalar.activation(eb_r[:, 16 * c + 8:16 * c + 16, c],
                                         ps_sc[:, 16 * c + 8:16 * c + 16],
                                         ACT.Exp, accum_out=e_7b)

            for c in range(B_LOC):
                fine = c in (0, B_LOC - 1)   # 4-tile chains for ramp/tail
                if c >= 1:
                    # ACT queue: previous example's exp runs while PE does
                    # this example's transposes/h-matmuls
                    exp_c(c - 1)
                for g in range(2):
                    base = 2048 * c + 1024 * g
                    j0 = 16 * c + 8 * g
                    ph = ps_h_pool.tile([128, 1024], f32, tag="ph")
                    if fine:
                        if c == B_LOC - 1:
                            # DVE casts for c7 quarters q0 / q2
                            cast(nc.vector, 112 + 8 * g, 116 + 8 * g)
                        transposes(j0, 1)
                        nc.tensor.matmul(ph[:, 0:512], lhsT=wc_bf,
                                         rhs=xT[:, base:base + 512])
                        nc.scalar.activation(ht[:, base:base + 512],
                                             ph[:, 0:512], ACT.Tanh,
                                             bias=b_sum)
                        transposes(j0 + 4, 1)
                        nc.tensor.matmul(ph[:, 512:1024], lhsT=wc_bf,
                                         rhs=xT[:, base + 512:base + 1024])
                        nc.scalar.activation(ht[:, base + 512:base + 1024],
                                             ph[:, 512:1024], ACT.Tanh,
                                             bias=b_sum)
                    else:
                        transposes(j0, 2)
                        nc.tensor.matmul(ph[:, 0:512], lhsT=wc_bf,
                                         rhs=xT[:, base:base + 512])
                        nc.tensor.matmul(ph[:, 512:1024], lhsT=wc_bf,
                                         rhs=xT[:, base + 512:base + 1024])
                        nc.scalar.activation(ht[:, base:base + 1024], ph,
                                             ACT.Tanh, bias=b_sum)
                # previous example's ctx rides between c's h and scores
                if c >= 1:
                    ctx_mms(c - 1)
                scores(16 * c, 8)
                scores(16 * c + 8, 8)

            # ---- tail: c7 epilogue, denominator, transpose/scale ----
            c = B_LOC - 1
            exp_c(c)
            for half in range(2):
                for i in range(8):
                    j = 16 * c + 8 * half + i
                    nc.tensor.matmul(ps_cx[:, 0:8], lhsT=x_bf_r[:, j],
                                     rhs=eb_r[:, j],
                                     start=(j == 0), stop=(j == N_TILES - 1))

            nc.vector.tensor_add(e_all[:, c:c + 1], e_7a, e_7b)
            nc.tensor.matmul(ps_cx[0:8, 448:449], lhsT=e_all, rhs=ones_col)

            nc.vector.tensor_copy(cxT_sb, ps_cx[:, 0:8])
            nc.tensor.transpose(ps_cx[0:8, 320:448], cxT_sb, ident_f)

            nc.vector.reciprocal(den_r, ps_cx[0:8, 448:449])
            nc.vector.tensor_scalar_mul(out_sb, ps_cx[0:8, 320:448], den_r)
            nc.sync.dma_start(out=out_d.ap(), in_=out_sb)

    nc.compile()
    return nc


def get_nc():
    global _nc
    if _nc is None:
        _nc = _build_nc()
    return _nc


def kernel(encoder_outputs, W1_w, W1_b, W2_w, W2_b, V_w, V_b):
    global LAST_RESULT
    from concourse.bass_utils import run_bass_kernel_spmd

    nc = get_nc()

    enc = np.ascontiguousarray(np.asarray(encoder_outputs, dtype=np.float32))
    rep = {
        "W1_w": np.ascontiguousarray(np.asarray(W1_w, np.float32)),
        "W1_b": np.ascontiguousarray(np.asarray(W1_b, np.float32).reshape(U, 1)),
        "W2_w": np.ascontiguousarray(np.asarray(W2_w, np.float32)),
        "W2_b": np.ascontiguousarray(np.asarray(W2_b, np.float32).reshape(U, 1)),
        "V_w": np.ascontiguousarray(np.asarray(V_w, np.float32).reshape(U, 1)),
    }
    in_maps = []
    for c in range(N_CORES):
        shard = enc[c * B_LOC:(c + 1) * B_LOC].reshape(B_LOC * T, D)
        in_maps.append({"encoder_outputs": np.ascontiguousarray(shard), **rep})

    trace = bool(int(os.environ.get("KERNEL_TRACE", "0")))
    LAST_RESULT = run_bass_kernel_spmd(
        nc, in_maps, core_ids=list(range(N_CORES)), trace=trace)
    out = np.concatenate(
        [LAST_RESULT.results[c]["out"] for c in range(N_CORES)], axis=0)
    return np.ascontiguousarray(out, dtype=np.float32)


# revision 9
# speedup vs baseline: 1.1654x; 1.1654x over previous
"""Additive-attention pooling kernel for 8 TRN2 NeuronCores.

reference:
    h     = tanh(x @ (W1+W2) + (b1+b2))      x: [B, T, D]
    score = h @ V + V_b                      [B, T, 1]
    attn  = softmax(score, axis=T)
    out   = sum_t attn * x                   [B, D]

Sharding: data-parallel over batch; each of the 8 cores gets B/8 = 8
examples (8 MB of fp32), weights replicated. No collectives.

Layout: token t = c*2048 + p*16 + i lands on partition p, tile i of
example c (token order within an example is permuted vs the reference;
softmax pooling is permutation-invariant).  Each partition reads
contiguous 2-8 KB runs from HBM.

v2 data path (vs the v1 SWDGE-cast design): x streams as plain f32
over the HARDWARE DGE (SP + ACT queues), which starts at ~4 us and
runs at HBM line rate, instead of the software-DGE cast path that was
desc-gen paced (~288 GB/s) and started at ~9 us behind the Q7
preamble.  f32 -> bf16 casts are split across DVE (c0, c1, part of
c7) and the otherwise-idle GPSIMD/Pool engine (c2..c6, rest of c7).
Identities are built on DVE so Q7 never blocks anything.

Per-core compute per 4-tile cluster: PE transposes x (xT) via the
identity; PE h-matmul with stationary Wc -> psum; ACT tanh(+bias) ->
hT sbuf; PE score matmul (stationary hT tile, 1-col rhs v) -> psum
column; ACT exp per example -> masked e-block column with accum_out
denominator; PE context matmul (stationary x tile, 8-col e-block rhs)
accumulating ctx^T [d, 8] across all 128 tiles in one psum bank.
PE emission is software-pipelined: transposes+h-matmuls of example c
run before scores of c (hiding the tanh latency), and ctx of c-1 is
interleaved into c's slot.  First and last examples run at 4-tile
granularity for ramp/tail latency; the middle at 8-tile.

V_b is omitted: softmax(score + c) == softmax(score) exactly.
Softmax runs without max-subtraction: |score| <= sum|V_u| ~ 9.
"""

import os

import numpy as np

B, T, D, U = 64, 2048, 128, 128
N_CORES = 8
B_LOC = B // N_CORES          # 8 examples per core
N_TILE_EX = T // 128          # 16 token-tiles per example
N_TILES = B_LOC * N_TILE_EX   # 128 tiles per core

_nc = None
LAST_RESULT = None


def _build_nc():
    import concourse.bass as bass  # noqa: F401
    import concourse.mybir as mybir
    import concourse.tile as tile
    from concourse import bacc

    f32 = mybir.dt.float32
    bf16 = mybir.dt.bfloat16
    ACT = mybir.ActivationFunctionType
    ALU = mybir.AluOpType

    nc = bacc.Bacc("TRN2", target_bir_lowering=False, debug=False,
                   num_devices=N_CORES)

    x_d = nc.declare_dram_parameter("encoder_outputs", [B_LOC * T, D], f32,
                                    isOutput=False)
    w1_d = nc.declare_dram_parameter("W1_w", [D, U], f32, isOutput=False)
    b1_d = nc.declare_dram_parameter("W1_b", [U, 1], f32, isOutput=False)
    w2_d = nc.declare_dram_parameter("W2_w", [D, U], f32, isOutput=False)
    b2_d = nc.declare_dram_parameter("W2_b", [U, 1], f32, isOutput=False)
    v_d = nc.declare_dram_parameter("V_w", [U, 1], f32, isOutput=False)
    out_d = nc.declare_dram_parameter("out", [B_LOC, D], f32, isOutput=True)

    # token = c*T + p*16 + i  ->  [c][p][i][d]; per-(c,p) HBM runs are 8 KB
    x_r = x_d.ap().rearrange("(c p i) d -> c p i d", c=B_LOC, p=128,
                             i=N_TILE_EX)

    def make_ident(eng, dst):
        eng.memset(dst, 0.0)
        eng.affine_select(out=dst, in_=dst, compare_op=ALU.not_equal,
                          fill=1.0, base=0, pattern=[[-1, dst.shape[1]]],
                          channel_multiplier=1)

    with tile.TileContext(nc) as tc:
        with (
            tc.tile_pool(name="consts", bufs=1) as consts,
            tc.tile_pool(name="big", bufs=1) as big,
            tc.tile_pool(name="ps_xt", bufs=2, space="PSUM") as ps_xt_pool,
            tc.tile_pool(name="ps_h", bufs=2, space="PSUM") as ps_h_pool,
            tc.tile_pool(name="ps_sc", bufs=1, space="PSUM") as ps_sc_pool,
            tc.tile_pool(name="ps_cx", bufs=1, space="PSUM") as ps_cx_pool,
        ):
            # ---- persistent buffers ----
            xf = big.tile([128, N_TILE_EX, 128], f32)       # c0 f32 staging
            x_bf = big.tile([128, N_TILES * 128], bf16)     # 4 MB  [t, d]
            xT = big.tile([128, N_TILES * 128], bf16)       # 4 MB  [d, t]
            ht = big.tile([128, N_TILES * 128], bf16)       # 4 MB tanh(h)^T
            eb = big.tile([128, N_TILES * 8], bf16)         # masked e-blocks
            e_all = big.tile([128, 8], f32)
            e_7a = big.tile([128, 1], f32)
            e_7b = big.tile([128, 1], f32)
            cxT_sb = big.tile([128, 8], f32)
            out_sb = big.tile([B_LOC, 128], f32)
            den_r = big.tile([B_LOC, 1], f32)

            x_bf_r = x_bf.rearrange("p (j d) -> p j d", d=128)
            eb_r = eb.rearrange("p (j e) -> p j e", e=8)

            ident = consts.tile([128, 128], bf16)
            ident_f = consts.tile([128, 128], f32)
            w1_sb = consts.tile([128, 128], f32)
            w2_sb = consts.tile([128, 128], f32)
            wc_bf = consts.tile([128, 128], bf16)   # [d, u] stationary
            b1_sb = consts.tile([128, 1], f32)
            b2_sb = consts.tile([128, 1], f32)
            b_sum = consts.tile([128, 1], f32)      # per-partition (=u) bias
            vf_sb = consts.tile([128, 1], f32)
            v_bf = consts.tile([128, 1], bf16)
            ones_col = consts.tile([128, 1], f32)
            act_warm = consts.tile([128, 1], f32)

            ps_sc = ps_sc_pool.tile([128, 128], f32)   # score col per tile
            ps_cx = ps_cx_pool.tile([128, 512], f32)

            # ================= DMA issue =================
            # Every queue pays a ~6.5us framework preamble before its first
            # instruction, so nothing moves before ~7.5us.  c0 rides HWDGE
            # f32 on the SP queue (fastest available start) + DVE casts;
            # c1..c7 ride SWDGE casting transfers issued on Q7 right after
            # the identity builds, landing bf16 at no compute-engine cost.
            nc.sync.dma_start(out=w1_sb, in_=w1_d.ap())
            nc.sync.dma_start(out=w2_sb, in_=w2_d.ap())
            for q in range(4):
                nc.sync.dma_start(out=xf[:, 4 * q:4 * (q + 1)],
                                  in_=x_r[0][:, 4 * q:4 * (q + 1)])
            # bias/v configs on the ACT queue (sits behind the hoisted
            # activation-table load; lands in time for the first tanh/score)
            nc.scalar.dma_start(out=b1_sb, in_=b1_d.ap())
            nc.scalar.dma_start(out=b2_sb, in_=b2_d.ap())
            nc.scalar.dma_start(out=vf_sb, in_=v_d.ap())

            # Q7: identities, then the SWDGE cast stream for c1..c7
            make_ident(nc.gpsimd, ident)
            make_ident(nc.gpsimd, ident_f)   # needed only at the tail
            for h in range(2):               # c1 in halves (chase grain)
                nc.gpsimd.dma_start(out=x_bf_r[:, 16 + 8 * h:24 + 8 * h],
                                    in_=x_r[1][:, 8 * h:8 * (h + 1)])
            for c in range(2, 7):            # bulk examples whole
                nc.gpsimd.dma_start(out=x_bf_r[:, 16 * c:16 * (c + 1)],
                                    in_=x_r[c])
            for q in range(4):               # last example in quarters
                nc.gpsimd.dma_start(
                    out=x_bf_r[:, 112 + 4 * q:116 + 4 * q],
                    in_=x_r[7][:, 4 * q:4 * (q + 1)])

            nc.vector.memset(ones_col, 1.0)
            # ACT: warm the exp/tanh table during the DMA ramp
            nc.scalar.activation(act_warm, ones_col, ACT.Exp)

            def cast(eng, lo, hi):     # xf -> x_bf, tile range [lo, hi)
                eng.tensor_copy(x_bf_r[:, lo:hi], xf[:, lo:hi])

            nc.vector.tensor_add(wc_bf, w1_sb, w2_sb)
            cast(nc.vector, 0, 4)
            nc.vector.tensor_add(b_sum, b1_sb, b2_sb)
            cast(nc.vector, 4, 8)
            nc.vector.tensor_copy(v_bf, vf_sb)
            cast(nc.vector, 8, 12)
            nc.vector.memset(eb, 0.0)
            cast(nc.vector, 12, 16)

            # ================= main pipeline =================
            # Per example c: PE does transposes + h-matmuls for both halves,
            # then ctx(c-1), then scores(c).  ACT does exp(c-1) then tanh(c).
            def transposes(j0, n):
                """n 4-tile clusters starting at tile j0 -> xT via psum."""
                for q in range(n):
                    pxt = ps_xt_pool.tile([128, 512], bf16, tag="pxt")
                    for r in range(4):
                        j = j0 + 4 * q + r
                        nc.tensor.transpose(
                            pxt[:, 128 * r:128 * (r + 1)],
                            x_bf_r[:, j], ident)
                    s = 128 * (j0 + 4 * q)
                    nc.vector.tensor_copy(xT[:, s:s + 512], pxt)

            def ctx_mms(c):
                for i in range(N_TILE_EX):
                    j = 16 * c + i
                    nc.tensor.matmul(ps_cx[:, 0:8], lhsT=x_bf_r[:, j],
                                     rhs=eb_r[:, j],
                                     start=(j == 0), stop=(j == N_TILES - 1))

            def scores(j0, n):
                for i in range(n):
                    j = j0 + i
                    nc.tensor.matmul(ps_sc[:, j:j + 1],
                                     lhsT=ht[:, 128 * j:128 * (j + 1)],
                                     rhs=v_bf)

            def exp_c(c):
                if c < B_LOC - 1:
                    nc.scalar.activation(eb_r[:, 16 * c:16 * c + 16, c],
                                         ps_sc[:, 16 * c:16 * c + 16],
                                         ACT.Exp,
                                         accum_out=e_all[:, c:c + 1])
                else:
                    nc.scalar.activation(eb_r[:, 16 * c:16 * c + 8, c],
                                         ps_sc[:, 16 * c:16 * c + 8],
                                         ACT.Exp, accum_out=e_7a)
                    nc.scalar.activation(eb_r[:, 16 * c + 8:16 * c + 16, c],
                                         ps_sc[:, 16 * c + 8:16 * c + 16],
                                         ACT.Exp, accum_out=e_7b)

            for c in range(B_LOC):
                fine = c in (0, B_LOC - 1)   # 4-tile chains for ramp/tail
                if c >= 1:
                    # ACT queue: previous example's exp runs while PE does
                    # this example's transposes/h-matmuls
                    exp_c(c - 1)
                for g in range(2):
                    base = 2048 * c + 1024 * g
                    j0 = 16 * c + 8 * g
                    ph = ps_h_pool.tile([128, 1024], f32, tag="ph")
                    if fine:
                        transposes(j0, 1)
                        nc.tensor.matmul(ph[:, 0:512], lhsT=wc_bf,
                                         rhs=xT[:, base:base + 512])
                        nc.scalar.activation(ht[:, base:base + 512],
                                             ph[:, 0:512], ACT.Tanh,
                                             bias=b_sum)
                        transposes(j0 + 4, 1)
                        nc.tensor.matmul(ph[:, 512:1024], lhsT=wc_bf,
                                         rhs=xT[:, base + 512:base + 1024])
                        nc.scalar.activation(ht[:, base + 512:base + 1024],
                                             ph[:, 512:1024], ACT.Tanh,
                                             bias=b_sum)
                    else:
                        transposes(j0, 2)
                        nc.tensor.matmul(ph[:, 0:512], lhsT=wc_bf,
                                         rhs=xT[:, base:base + 512])
                        nc.tensor.matmul(ph[:, 512:1024], lhsT=wc_bf,
                                         rhs=xT[:, base + 512:base + 1024])
                        nc.scalar.activation(ht[:, base:base + 1024], ph,
                                             ACT.Tanh, bias=b_sum)
                # previous example's ctx rides between c's h and scores
                if c >= 1:
                    ctx_mms(c - 1)
                scores(16 * c, 8)
                scores(16 * c + 8, 8)

            # ---- tail: c7 epilogue, denominator, transpose/scale ----
            c = B_LOC - 1
            exp_c(c)
            for half in range(2):
                for i in range(8):
                    j = 16 * c + 8 * half + i
                    nc.tensor.matmul(ps_cx[:, 0:8], lhsT=x_bf_r[:, j],
                                     rhs=eb_r[:, j],
                                     start=(j == 0), stop=(j == N_TILES - 1))

            nc.vector.tensor_add(e_all[:, c:c + 1], e_7a, e_7b)
            nc.tensor.matmul(ps_cx[0:8, 448:449], lhsT=e_all, rhs=ones_col)

            nc.vector.tensor_copy(cxT_sb, ps_cx[:, 0:8])
            nc.tensor.transpose(ps_cx[0:8, 320:448], cxT_sb, ident_f)

            nc.vector.reciprocal(den_r, ps_cx[0:8, 448:449])
            nc.vector.tensor_scalar_mul(out_sb, ps_cx[0:8, 320:448], den_r)
            nc.sync.dma_start(out=out_d.ap(), in_=out_sb)

    nc.compile()
    return nc


def get_nc():
    global _nc
    if _nc is None:
        _nc = _build_nc()
    return _nc


def kernel(encoder_outputs, W1_w, W1_b, W2_w, W2_b, V_w, V_b):
    global LAST_RESULT
    from concourse.bass_utils import run_bass_kernel_spmd

    nc = get_nc()

    enc = np.ascontiguousarray(np.asarray(encoder_outputs, dtype=np.float32))
    rep = {
        "W1_w": np.ascontiguousarray(np.asarray(W1_w, np.float32)),
        "W1_b": np.ascontiguousarray(np.asarray(W1_b, np.float32).reshape(U, 1)),
        "W2_w": np.ascontiguousarray(np.asarray(W2_w, np.float32)),
        "W2_b": np.ascontiguousarray(np.asarray(W2_b, np.float32).reshape(U, 1)),
        "V_w": np.ascontiguousarray(np.asarray(V_w, np.float32).reshape(U, 1)),
    }
    in_maps = []
    for c in range(N_CORES):
        shard = enc[c * B_LOC:(c + 1) * B_LOC].reshape(B_LOC * T, D)
        in_maps.append({"encoder_outputs": np.ascontiguousarray(shard), **rep})

    trace = bool(int(os.environ.get("KERNEL_TRACE", "0")))
    LAST_RESULT = run_bass_kernel_spmd(
        nc, in_maps, core_ids=list(range(N_CORES)), trace=trace)
    out = np.concatenate(
        [LAST_RESULT.results[c]["out"] for c in range(N_CORES)], axis=0)
    return np.ascontiguousarray(out, dtype=np.float32)


# revision 11
# speedup vs baseline: 1.2066x; 1.0354x over previous
"""Additive-attention pooling kernel for 8 TRN2 NeuronCores.

reference:
    h     = tanh(x @ (W1+W2) + (b1+b2))      x: [B, T, D]
    score = h @ V + V_b                      [B, T, 1]
    attn  = softmax(score, axis=T)
    out   = sum_t attn * x                   [B, D]

Sharding: data-parallel over batch; each of the 8 cores gets B/8 = 8
examples (8 MB of fp32), weights replicated. No collectives.

Layout: token t = c*2048 + p*16 + i lands on partition p, tile i of
example c (token order within an example is permuted vs the reference;
softmax pooling is permutation-invariant).  Each partition reads
contiguous 2-8 KB runs from HBM.

v2 data path (vs the v1 SWDGE-cast design): x streams as plain f32
over the HARDWARE DGE (SP + ACT queues), which starts at ~4 us and
runs at HBM line rate, instead of the software-DGE cast path that was
desc-gen paced (~288 GB/s) and started at ~9 us behind the Q7
preamble.  f32 -> bf16 casts are split across DVE (c0, c1, part of
c7) and the otherwise-idle GPSIMD/Pool engine (c2..c6, rest of c7).
Identities are built on DVE so Q7 never blocks anything.

Per-core compute per 4-tile cluster: PE transposes x (xT) via the
identity; PE h-matmul with stationary Wc -> psum; ACT tanh(+bias) ->
hT sbuf; PE score matmul (stationary hT tile, 1-col rhs v) -> psum
column; ACT exp per example -> masked e-block column with accum_out
denominator; PE context matmul (stationary x tile, 8-col e-block rhs)
accumulating ctx^T [d, 8] across all 128 tiles in one psum bank.
PE emission is software-pipelined: transposes+h-matmuls of example c
run before scores of c (hiding the tanh latency), and ctx of c-1 is
interleaved into c's slot.  First and last examples run at 4-tile
granularity for ramp/tail latency; the middle at 8-tile.

V_b is omitted: softmax(score + c) == softmax(score) exactly.
Softmax runs without max-subtraction: |score| <= sum|V_u| ~ 9.
"""

import os

import numpy as np

B, T, D, U = 64, 2048, 128, 128
N_CORES = 8
B_LOC = B // N_CORES          # 8 examples per core
N_TILE_EX = T // 128          # 16 token-tiles per example
N_TILES = B_LOC * N_TILE_EX   # 128 tiles per core

_nc = None
LAST_RESULT = None


def _build_nc():
    import concourse.bass as bass  # noqa: F401
    import concourse.mybir as mybir
    import concourse.tile as tile
    from concourse import bacc

    f32 = mybir.dt.float32
    bf16 = mybir.dt.bfloat16
    ACT = mybir.ActivationFunctionType
    ALU = mybir.AluOpType

    nc = bacc.Bacc("TRN2", target_bir_lowering=False, debug=False,
                   num_devices=N_CORES)

    x_d = nc.declare_dram_parameter("encoder_outputs", [B_LOC * T, D], f32,
                                    isOutput=False)
    w1_d = nc.declare_dram_parameter("W1_w", [D, U], f32, isOutput=False)
    b1_d = nc.declare_dram_parameter("W1_b", [U, 1], f32, isOutput=False)
    w2_d = nc.declare_dram_parameter("W2_w", [D, U], f32, isOutput=False)
    b2_d = nc.declare_dram_parameter("W2_b", [U, 1], f32, isOutput=False)
    v_d = nc.declare_dram_parameter("V_w", [U, 1], f32, isOutput=False)
    out_d = nc.declare_dram_parameter("out", [B_LOC, D], f32, isOutput=True)

    # token = c*T + p*16 + i  ->  [c][p][i][d]; per-(c,p) HBM runs are 8 KB
    x_r = x_d.ap().rearrange("(c p i) d -> c p i d", c=B_LOC, p=128,
                             i=N_TILE_EX)

    def make_ident(eng, dst):
        eng.memset(dst, 0.0)
        eng.affine_select(out=dst, in_=dst, compare_op=ALU.not_equal,
                          fill=1.0, base=0, pattern=[[-1, dst.shape[1]]],
                          channel_multiplier=1)

    with tile.TileContext(nc) as tc:
        with (
            tc.tile_pool(name="consts", bufs=1) as consts,
            tc.tile_pool(name="big", bufs=1) as big,
            tc.tile_pool(name="ps_xt", bufs=2, space="PSUM") as ps_xt_pool,
            tc.tile_pool(name="ps_h", bufs=2, space="PSUM") as ps_h_pool,
            tc.tile_pool(name="ps_sc", bufs=1, space="PSUM") as ps_sc_pool,
            tc.tile_pool(name="ps_cx", bufs=1, space="PSUM") as ps_cx_pool,
        ):
            # ---- persistent buffers ----
            x_bf = big.tile([128, N_TILES * 128], bf16)     # 4 MB  [t, d]
            xT = big.tile([128, N_TILES * 128], bf16)       # 4 MB  [d, t]
            ht = big.tile([128, N_TILES * 128], bf16)       # 4 MB tanh(h)^T
            eb = big.tile([128, N_TILES * 8], bf16)         # masked e-blocks
            e_all = big.tile([128, 8], f32)
            e_7a = big.tile([128, 1], f32)
            e_7b = big.tile([128, 1], f32)
            cxT_sb = big.tile([128, 8], f32)
            out_sb = big.tile([B_LOC, 128], f32)
            den_r = big.tile([B_LOC, 1], f32)

            x_bf_r = x_bf.rearrange("p (j d) -> p j d", d=128)
            eb_r = eb.rearrange("p (j e) -> p j e", e=8)

            ident = consts.tile([128, 128], bf16)
            ident_f = consts.tile([128, 128], f32)
            w1_sb = consts.tile([128, 128], f32)
            w2_sb = consts.tile([128, 128], f32)
            wc_bf = consts.tile([128, 128], bf16)   # [d, u] stationary
            b1_sb = consts.tile([128, 1], f32)
            b2_sb = consts.tile([128, 1], f32)
            b_sum = consts.tile([128, 1], f32)      # per-partition (=u) bias
            vf_sb = consts.tile([128, 1], f32)
            v_bf = consts.tile([128, 1], bf16)
            ones_col = consts.tile([128, 1], f32)
            act_warm = consts.tile([128, 1], f32)

            ps_sc = ps_sc_pool.tile([128, 128], f32)   # score col per tile
            ps_cx = ps_cx_pool.tile([128, 512], f32)

            # ================= DMA issue =================
            # Every queue pays a ~6.5us framework preamble, so nothing moves
            # before ~7.5us.  ALL of x rides ONE SWDGE casting stream on Q7
            # in consumption order (f32 HBM -> bf16 SBUF inside the DMA
            # engines; a competing HWDGE x stream just steals engine slots
            # from the head of this one — measured in v3).  First/last
            # examples are quartered for ramp/tail grain.
            make_ident(nc.gpsimd, ident)     # ~0.55us, before the stream
            for q in range(4):               # c0 quarters
                nc.gpsimd.dma_start(out=x_bf_r[:, 4 * q:4 * (q + 1)],
                                    in_=x_r[0][:, 4 * q:4 * (q + 1)])
            for h in range(2):               # c1 halves
                nc.gpsimd.dma_start(out=x_bf_r[:, 16 + 8 * h:24 + 8 * h],
                                    in_=x_r[1][:, 8 * h:8 * (h + 1)])
            for c in range(2, 7):            # bulk examples whole
                nc.gpsimd.dma_start(out=x_bf_r[:, 16 * c:16 * (c + 1)],
                                    in_=x_r[c])
            for q in range(4):               # last example in quarters
                nc.gpsimd.dma_start(
                    out=x_bf_r[:, 112 + 4 * q:116 + 4 * q],
                    in_=x_r[7][:, 4 * q:4 * (q + 1)])
            make_ident(nc.gpsimd, ident_f)   # needed only at the tail

            # weights on the SP HWDGE (idle engines before the stream arms)
            nc.sync.dma_start(out=w1_sb, in_=w1_d.ap())
            nc.sync.dma_start(out=w2_sb, in_=w2_d.ap())
            # bias/v configs on the ACT queue (behind the hoisted
            # activation-table load; land in time for first tanh/score)
            nc.scalar.dma_start(out=b1_sb, in_=b1_d.ap())
            nc.scalar.dma_start(out=b2_sb, in_=b2_d.ap())
            nc.scalar.dma_start(out=vf_sb, in_=v_d.ap())

            nc.vector.memset(ones_col, 1.0)
            # ACT: warm the exp/tanh table during the DMA ramp
            nc.scalar.activation(act_warm, ones_col, ACT.Exp)

            nc.vector.tensor_add(wc_bf, w1_sb, w2_sb)
            nc.vector.tensor_add(b_sum, b1_sb, b2_sb)
            nc.vector.tensor_copy(v_bf, vf_sb)
            nc.vector.memset(eb, 0.0)

            # ================= main pipeline =================
            # Per example c: PE does transposes + h-matmuls for both halves,
            # then ctx(c-1), then scores(c).  ACT does exp(c-1) then tanh(c).
            def transposes(j0, n):
                """n 4-tile clusters starting at tile j0 -> xT via psum."""
                for q in range(n):
                    pxt = ps_xt_pool.tile([128, 512], bf16, tag="pxt")
                    for r in range(4):
                        j = j0 + 4 * q + r
                        nc.tensor.transpose(
                            pxt[:, 128 * r:128 * (r + 1)],
                            x_bf_r[:, j], ident)
                    s = 128 * (j0 + 4 * q)
                    nc.vector.tensor_copy(xT[:, s:s + 512], pxt)

            def ctx_mms(c):
                for i in range(N_TILE_EX):
                    j = 16 * c + i
                    nc.tensor.matmul(ps_cx[:, 0:8], lhsT=x_bf_r[:, j],
                                     rhs=eb_r[:, j],
                                     start=(j == 0), stop=(j == N_TILES - 1))

            def scores(j0, n):
                for i in range(n):
                    j = j0 + i
                    nc.tensor.matmul(ps_sc[:, j:j + 1],
                                     lhsT=ht[:, 128 * j:128 * (j + 1)],
                                     rhs=v_bf)

            def exp_c(c):
                if c < B_LOC - 1:
                    nc.scalar.activation(eb_r[:, 16 * c:16 * c + 16, c],
                                         ps_sc[:, 16 * c:16 * c + 16],
                                         ACT.Exp,
                                         accum_out=e_all[:, c:c + 1])
                else:
                    nc.scalar.activation(eb_r[:, 16 * c:16 * c + 8, c],
                                         ps_sc[:, 16 * c:16 * c + 8],
                                         ACT.Exp, accum_out=e_7a)
                    nc.scalar.activation(eb_r[:, 16 * c + 8:16 * c + 16, c],
                                         ps_sc[:, 16 * c + 8:16 * c + 16],
                                         ACT.Exp, accum_out=e_7b)

            for c in range(B_LOC):
                fine = c in (0, B_LOC - 1)   # 4-tile chains for ramp/tail
                if c >= 1:
                    # ACT queue: previous example's exp runs while PE does
                    # this example's transposes/h-matmuls
                    exp_c(c - 1)
                for g in range(2):
                    base = 2048 * c + 1024 * g
                    j0 = 16 * c + 8 * g
                    ph = ps_h_pool.tile([128, 1024], f32, tag="ph")
                    if fine:
                        transposes(j0, 1)
                        nc.tensor.matmul(ph[:, 0:512], lhsT=wc_bf,
                                         rhs=xT[:, base:base + 512])
                        nc.scalar.activation(ht[:, base:base + 512],
                                             ph[:, 0:512], ACT.Tanh,
                                             bias=b_sum)
                        transposes(j0 + 4, 1)
                        nc.tensor.matmul(ph[:, 512:1024], lhsT=wc_bf,
                                         rhs=xT[:, base + 512:base + 1024])
                        nc.scalar.activation(ht[:, base + 512:base + 1024],
                                             ph[:, 512:1024], ACT.Tanh,
                                             bias=b_sum)
                    else:
                        transposes(j0, 2)
                        nc.tensor.matmul(ph[:, 0:512], lhsT=wc_bf,
                                         rhs=xT[:, base:base + 512])
                        nc.tensor.matmul(ph[:, 512:1024], lhsT=wc_bf,
                                         rhs=xT[:, base + 512:base + 1024])
                        nc.scalar.activation(ht[:, base:base + 1024], ph,
                                             ACT.Tanh, bias=b_sum)
                # previous example's ctx rides between c's h and scores
                if c >= 1:
                    ctx_mms(c - 1)
                scores(16 * c, 8)
                scores(16 * c + 8, 8)

            # ---- tail: c7 epilogue, denominator, transpose/scale ----
            c = B_LOC - 1
            exp_c(c)
            for half in range(2):
                for i in range(8):
                    j = 16 * c + 8 * half + i
                    nc.tensor.matmul(ps_cx[:, 0:8], lhsT=x_bf_r[:, j],
                                     rhs=eb_r[:, j],
                                     start=(j == 0), stop=(j == N_TILES - 1))

            nc.vector.tensor_add(e_all[:, c:c + 1], e_7a, e_7b)
            nc.tensor.matmul(ps_cx[0:8, 448:449], lhsT=e_all, rhs=ones_col)

            nc.vector.tensor_copy(cxT_sb, ps_cx[:, 0:8])
            nc.tensor.transpose(ps_cx[0:8, 320:448], cxT_sb, ident_f)

            nc.vector.reciprocal(den_r, ps_cx[0:8, 448:449])
            nc.vector.tensor_scalar_mul(out_sb, ps_cx[0:8, 320:448], den_r)
            nc.sync.dma_start(out=out_d.ap(), in_=out_sb)

    nc.compile()
    return nc


def get_nc():
    global _nc
    if _nc is None:
        _nc = _build_nc()
    return _nc


def kernel(encoder_outputs, W1_w, W1_b, W2_w, W2_b, V_w, V_b):
    global LAST_RESULT
    from concourse.bass_utils import run_bass_kernel_spmd

    nc = get_nc()

    enc = np.ascontiguousarray(np.asarray(encoder_outputs, dtype=np.float32))
    rep = {
        "W1_w": np.ascontiguousarray(np.asarray(W1_w, np.float32)),
        "W1_b": np.ascontiguousarray(np.asarray(W1_b, np.float32).reshape(U, 1)),
        "W2_w": np.ascontiguousarray(np.asarray(W2_w, np.float32)),
        "W2_b": np.ascontiguousarray(np.asarray(W2_b, np.float32).reshape(U, 1)),
        "V_w": np.ascontiguousarray(np.asarray(V_w, np.float32).reshape(U, 1)),
    }
    in_maps = []
    for c in range(N_CORES):
        shard = enc[c * B_LOC:(c + 1) * B_LOC].reshape(B_LOC * T, D)
        in_maps.append({"encoder_outputs": np.ascontiguousarray(shard), **rep})

    trace = bool(int(os.environ.get("KERNEL_TRACE", "0")))
    LAST_RESULT = run_bass_kernel_spmd(
        nc, in_maps, core_ids=list(range(N_CORES)), trace=trace)
    out = np.concatenate(
        [LAST_RESULT.results[c]["out"] for c in range(N_CORES)], axis=0)
    return np.ascontiguousarray(out, dtype=np.float32)


# revision 19
# speedup vs baseline: 1.4173x; 1.1747x over previous
"""Additive-attention pooling kernel for 8 TRN2 NeuronCores.

reference:
    h     = tanh(x @ (W1+W2) + (b1+b2))      x: [B, T, D]
    score = h @ V + V_b                      [B, T, 1]
    attn  = softmax(score, axis=T)
    out   = sum_t attn * x                   [B, D]

Sharding: data-parallel over batch; each of the 8 cores gets B/8 = 8
examples (8 MB of fp32), weights replicated. No collectives.

Layout: token t = c*2048 + p*16 + i lands on partition p, tile i of
example c (token order within an example is permuted vs the reference;
softmax pooling is permutation-invariant).  Each partition reads
contiguous 2-8 KB runs from HBM.

v2 data path (vs the v1 SWDGE-cast design): x streams as plain f32
over the HARDWARE DGE (SP + ACT queues), which starts at ~4 us and
runs at HBM line rate, instead of the software-DGE cast path that was
desc-gen paced (~288 GB/s) and started at ~9 us behind the Q7
preamble.  f32 -> bf16 casts are split across DVE (c0, c1, part of
c7) and the otherwise-idle GPSIMD/Pool engine (c2..c6, rest of c7).
Identities are built on DVE so Q7 never blocks anything.

Per-core compute per 4-tile cluster: PE transposes x (xT) via the
identity; PE h-matmul with stationary Wc -> psum; ACT tanh(+bias) ->
hT sbuf; PE score matmul (stationary hT tile, 1-col rhs v) -> psum
column; ACT exp per example -> masked e-block column with accum_out
denominator; PE context matmul (stationary x tile, 8-col e-block rhs)
accumulating ctx^T [d, 8] across all 128 tiles in one psum bank.
PE emission is software-pipelined: transposes+h-matmuls of example c
run before scores of c (hiding the tanh latency), and ctx of c-1 is
interleaved into c's slot.  First and last examples run at 4-tile
granularity for ramp/tail latency; the middle at 8-tile.

V_b is omitted: softmax(score + c) == softmax(score) exactly.
Softmax runs without max-subtraction: |score| <= sum|V_u| ~ 9.
"""

import os

import numpy as np

B, T, D, U = 64, 2048, 128, 128
N_CORES = 8
B_LOC = B // N_CORES          # 8 examples per core
N_TILE_EX = T // 128          # 16 token-tiles per example
N_TILES = B_LOC * N_TILE_EX   # 128 tiles per core

_nc = None
LAST_RESULT = None


def _build_nc():
    import concourse.bass as bass  # noqa: F401
    import concourse.mybir as mybir
    import concourse.tile as tile
    from concourse import bacc

    f32 = mybir.dt.float32
    bf16 = mybir.dt.bfloat16
    ACT = mybir.ActivationFunctionType
    ALU = mybir.AluOpType

    nc = bacc.Bacc("TRN2", target_bir_lowering=False, debug=False,
                   num_devices=N_CORES)

    x_d = nc.declare_dram_parameter("encoder_outputs", [B_LOC * T, D], f32,
                                    isOutput=False)
    w1_d = nc.declare_dram_parameter("W1_w", [D, U], f32, isOutput=False)
    b1_d = nc.declare_dram_parameter("W1_b", [U, 1], f32, isOutput=False)
    w2_d = nc.declare_dram_parameter("W2_w", [D, U], f32, isOutput=False)
    b2_d = nc.declare_dram_parameter("W2_b", [U, 1], f32, isOutput=False)
    v_d = nc.declare_dram_parameter("V_w", [U, 1], f32, isOutput=False)
    out_d = nc.declare_dram_parameter("out", [B_LOC, D], f32, isOutput=True)

    # token = c*T + p*16 + i  ->  [c][p][i][d]; per-(c,p) HBM runs are 8 KB
    x_r = x_d.ap().rearrange("(c p i) d -> c p i d", c=B_LOC, p=128,
                             i=N_TILE_EX)

    def make_ident(eng, dst):
        eng.memset(dst, 0.0)
        eng.affine_select(out=dst, in_=dst, compare_op=ALU.not_equal,
                          fill=1.0, base=0, pattern=[[-1, dst.shape[1]]],
                          channel_multiplier=1)

    with tile.TileContext(nc) as tc:
        with (
            tc.tile_pool(name="consts", bufs=1) as consts,
            tc.tile_pool(name="big", bufs=1) as big,
            tc.tile_pool(name="ps_xt", bufs=2, space="PSUM") as ps_xt_pool,
            tc.tile_pool(name="ps_h", bufs=2, space="PSUM") as ps_h_pool,
            tc.tile_pool(name="ps_sc", bufs=1, space="PSUM") as ps_sc_pool,
            tc.tile_pool(name="ps_cx", bufs=1, space="PSUM") as ps_cx_pool,
        ):
            # ---- persistent buffers ----
            x_bf = big.tile([128, N_TILES * 128], bf16)     # 4 MB  [t, d]
            xT = big.tile([128, N_TILES * 128], bf16)       # 4 MB  [d, t]
            ht = big.tile([128, N_TILES * 128], bf16)       # 4 MB tanh(h)^T
            eb = big.tile([128, N_TILES * 8], bf16)         # masked e-blocks
            e_all = big.tile([128, 8], f32)
            e_7a = big.tile([128, 1], f32)
            e_7b = big.tile([128, 1], f32)
            cxT_sb = big.tile([128, 8], f32)
            out_sb = big.tile([B_LOC, 128], f32)
            den_r = big.tile([B_LOC, 1], f32)

            x_bf_r = x_bf.rearrange("p (j d) -> p j d", d=128)
            eb_r = eb.rearrange("p (j e) -> p j e", e=8)

            ident = consts.tile([128, 128], bf16)
            ident_f = consts.tile([128, 128], f32)
            w1_sb = consts.tile([128, 128], f32)
            w2_sb = consts.tile([128, 128], f32)
            wc_bf = consts.tile([128, 128], bf16)   # [d, u] stationary
            b1_sb = consts.tile([128, 1], f32)
            b2_sb = consts.tile([128, 1], f32)
            b_sum = consts.tile([128, 1], f32)      # per-partition (=u) bias
            vf_sb = consts.tile([128, 1], f32)
            v_bf = consts.tile([128, 1], bf16)
            ones_col = consts.tile([128, 1], f32)
            act_warm = consts.tile([128, 1], f32)

            ps_sc = ps_sc_pool.tile([128, 128], f32)   # score col per tile
            ps_cx = ps_cx_pool.tile([128, 512], f32)

            # ================= DMA issue =================
            # Every queue pays a ~6.5us framework preamble, so nothing moves
            # before ~7.5us.  ALL of x rides ONE SWDGE casting stream on Q7
            # in consumption order (f32 HBM -> bf16 SBUF inside the DMA
            # engines; a competing HWDGE x stream just steals engine slots
            # from the head of this one — measured in v3).  The stream leads
            # with fine c0 chunks; the bf16 identity builds while c0's first
            # bytes are in flight.
            x_chunks = [(0, 2), (2, 2), None,        # c0: 2+2, then ident
                        (4, 4), (8, 8),              # c0 rest
                        (16, 8), (24, 8)]            # c1 halves
            x_chunks += [(16 * c, 16) for c in range(2, 7)]
            x_chunks += [(112 + 4 * q, 4) for q in range(4)]  # c7 quarters
            for item in x_chunks:
                if item is None:
                    make_ident(nc.gpsimd, ident)
                    continue
                lo, w = item
                src_c, src_lo = lo // 16, lo % 16
                nc.gpsimd.dma_start(
                    out=x_bf_r[:, lo:lo + w],
                    in_=x_r[src_c][:, src_lo:src_lo + w])
            make_ident(nc.gpsimd, ident_f)   # needed only at the tail

            # weights on the SP HWDGE (idle engines before the stream arms)
            nc.sync.dma_start(out=w1_sb, in_=w1_d.ap())
            nc.sync.dma_start(out=w2_sb, in_=w2_d.ap())
            # bias/v configs on the ACT queue (behind the hoisted
            # activation-table load; land in time for first tanh/score)
            nc.scalar.dma_start(out=b1_sb, in_=b1_d.ap())
            nc.scalar.dma_start(out=b2_sb, in_=b2_d.ap())
            nc.scalar.dma_start(out=vf_sb, in_=v_d.ap())

            # scratch for the PE HAM warm-up (memset first so the dummy
            # matmuls never read uninitialized SBUF)
            scratch = consts.tile([128, 512], bf16)
            nc.vector.memset(scratch, 0.5)
            nc.vector.memset(ones_col, 1.0)
            # ACT: warm the exp/tanh table during the DMA ramp
            nc.scalar.activation(act_warm, ones_col, ACT.Exp)

            # PE warm-up: real (non-transpose) matmuls in the otherwise-dead
            # window before c0 lands, to pull the HAM full-clock flip
            # earlier.  They chain WAW on one psum tile; done before the
            # first real transpose needs the pool.
            ph_warm = ps_h_pool.tile([128, 1024], f32, tag="ph")
            for _ in range(8):
                nc.tensor.matmul(ph_warm[:, 0:512], lhsT=scratch[:, 0:128],
                                 rhs=scratch)

            nc.vector.tensor_add(wc_bf, w1_sb, w2_sb)
            nc.vector.tensor_add(b_sum, b1_sb, b2_sb)
            nc.vector.tensor_copy(v_bf, vf_sb)
            nc.vector.memset(eb, 0.0)

            # ================= main pipeline =================
            # Per example c: PE does transposes + h-matmuls for both halves,
            # then ctx(c-1), then scores(c).  ACT does exp(c-1) then tanh(c).
            def transposes(j0, w):
                """One w-tile cluster starting at tile j0 -> xT via psum.
                w=8 amortizes the DVE psum-drain copy; w=4 for ramp/tail."""
                pxt = ps_xt_pool.tile([128, 128 * w], bf16, tag="pxt")
                for r in range(w):
                    nc.tensor.transpose(
                        pxt[:, 128 * r:128 * (r + 1)],
                        x_bf_r[:, j0 + r], ident)
                s = 128 * j0
                nc.vector.tensor_copy(xT[:, s:s + 128 * w], pxt)

            def ctx_mms(c):
                for i in range(N_TILE_EX):
                    j = 16 * c + i
                    nc.tensor.matmul(ps_cx[:, 0:8], lhsT=x_bf_r[:, j],
                                     rhs=eb_r[:, j],
                                     start=(j == 0), stop=(j == N_TILES - 1))

            def scores(j0, n):
                for i in range(n):
                    j = j0 + i
                    nc.tensor.matmul(ps_sc[:, j:j + 1],
                                     lhsT=ht[:, 128 * j:128 * (j + 1)],
                                     rhs=v_bf)

            def exp_c(c):
                nc.scalar.activation(eb_r[:, 16 * c:16 * c + 16, c],
                                     ps_sc[:, 16 * c:16 * c + 16],
                                     ACT.Exp, accum_out=e_all[:, c:c + 1])

            def h_mm(ph, base, lo, hi):
                nc.tensor.matmul(ph[:, lo:hi], lhsT=wc_bf,
                                 rhs=xT[:, base + lo:base + hi])

            for c in range(B_LOC - 1):
                if c >= 1:
                    # ACT queue: previous example's exp runs while PE does
                    # this example's transposes/h-matmuls
                    exp_c(c - 1)
                for g in range(2):
                    base = 2048 * c + 1024 * g
                    j0 = 16 * c + 8 * g
                    ph = ps_h_pool.tile([128, 1024], f32, tag="ph")
                    if c == 0:   # 4-tile chains for the ramp
                        transposes(j0, 4)
                        h_mm(ph, base, 0, 512)
                        nc.scalar.activation(ht[:, base:base + 512],
                                             ph[:, 0:512], ACT.Tanh,
                                             bias=b_sum)
                        transposes(j0 + 4, 4)
                        h_mm(ph, base, 512, 1024)
                        nc.scalar.activation(ht[:, base + 512:base + 1024],
                                             ph[:, 512:1024], ACT.Tanh,
                                             bias=b_sum)
                    else:
                        transposes(j0, 8)
                        h_mm(ph, base, 0, 512)
                        h_mm(ph, base, 512, 1024)
                        nc.scalar.activation(ht[:, base:base + 1024], ph,
                                             ACT.Tanh, bias=b_sum)
                # previous example's ctx rides between c's h and scores
                if c >= 1:
                    ctx_mms(c - 1)
                scores(16 * c, 8)
                scores(16 * c + 8, 8)

            # ---- last example: fully pipelined per-quarter tail ----
            c = B_LOC - 1
            e7q = [e_7a, e_7b,
                   big.tile([128, 1], f32, name="e_7c"),
                   big.tile([128, 1], f32, name="e_7d")]
            phs = []

            def c7_quarter(q):
                j0 = 112 + 4 * q
                if q % 2 == 0:
                    phs.append(ps_h_pool.tile([128, 1024], f32, tag="ph",
                                              name="ph7"))
                ph = phs[-1]
                lo = 512 * (q % 2)
                transposes(j0, 4)
                nc.tensor.matmul(ph[:, lo:lo + 512], lhsT=wc_bf,
                                 rhs=xT[:, 128 * j0:128 * j0 + 512])
                nc.scalar.activation(ht[:, 128 * j0:128 * j0 + 512],
                                     ph[:, lo:lo + 512], ACT.Tanh,
                                     bias=b_sum)

            def sc7(q):
                scores(112 + 4 * q, 4)

            def exp7(q):
                nc.scalar.activation(
                    eb_r[:, 112 + 4 * q:116 + 4 * q, c],
                    ps_sc[:, 112 + 4 * q:116 + 4 * q],
                    ACT.Exp, accum_out=e7q[q])

            def ctx7(q):
                for i in range(4):
                    j = 112 + 4 * q + i
                    nc.tensor.matmul(ps_cx[:, 0:8], lhsT=x_bf_r[:, j],
                                     rhs=eb_r[:, j],
                                     start=(j == 0), stop=(j == N_TILES - 1))

            exp_c(c - 1)
            c7_quarter(0)
            c7_quarter(1)
            ctx_mms(c - 1)
            sc7(0); exp7(0)
            c7_quarter(2)
            sc7(1); exp7(1)
            ctx7(0)
            c7_quarter(3)
            sc7(2); exp7(2)
            ctx7(1)
            sc7(3); exp7(3)
            ctx7(2)
            ctx7(3)

            # ---- denominator + final transpose/scale ----
            s01 = big.tile([128, 1], f32)
            s23 = big.tile([128, 1], f32)
            nc.vector.tensor_add(s01, e7q[0], e7q[1])
            nc.vector.tensor_add(s23, e7q[2], e7q[3])
            nc.vector.tensor_add(e_all[:, c:c + 1], s01, s23)
            nc.tensor.matmul(ps_cx[0:8, 448:449], lhsT=e_all, rhs=ones_col)

            nc.vector.tensor_copy(cxT_sb, ps_cx[:, 0:8])
            nc.tensor.transpose(ps_cx[0:8, 320:448], cxT_sb, ident_f)

            nc.vector.reciprocal(den_r, ps_cx[0:8, 448:449])
            nc.vector.tensor_scalar_mul(out_sb, ps_cx[0:8, 320:448], den_r)
            nc.sync.dma_start(out=out_d.ap(), in_=out_sb)

    nc.compile()
    return nc


def get_nc():
    global _nc
    if _nc is None:
        _nc = _build_nc()
    return _nc


def kernel(encoder_outputs, W1_w, W1_b, W2_w, W2_b, V_w, V_b):
    global LAST_RESULT
    from concourse.bass_utils import run_bass_kernel_spmd

    nc = get_nc()

    enc = np.ascontiguousarray(np.asarray(encoder_outputs, dtype=np.float32))
    rep = {
        "W1_w": np.ascontiguousarray(np.asarray(W1_w, np.float32)),
        "W1_b": np.ascontiguousarray(np.asarray(W1_b, np.float32).reshape(U, 1)),
        "W2_w": np.ascontiguousarray(np.asarray(W2_w, np.float32)),
        "W2_b": np.ascontiguousarray(np.asarray(W2_b, np.float32).reshape(U, 1)),
        "V_w": np.ascontiguousarray(np.asarray(V_w, np.float32).reshape(U, 1)),
    }
    in_maps = []
    for c in range(N_CORES):
        shard = enc[c * B_LOC:(c + 1) * B_LOC].reshape(B_LOC * T, D)
        in_maps.append({"encoder_outputs": np.ascontiguousarray(shard), **rep})

    trace = bool(int(os.environ.get("KERNEL_TRACE", "0")))
    LAST_RESULT = run_bass_kernel_spmd(
        nc, in_maps, core_ids=list(range(N_CORES)), trace=trace)
    out = np.concatenate(
        [LAST_RESULT.results[c]["out"] for c in range(N_CORES)], axis=0)
    return np.ascontiguousarray(out, dtype=np.float32)
